# revision 2
# baseline (speedup 1.0000x reference)
import numpy as np
import concourse.bass as bass
import concourse.bacc as bacc
import concourse.mybir as mybir
from concourse.tile import TileContext
from concourse.bass_utils import run_bass_kernel_spmd

B, HID = 4096, 512
NR, NB = 32, 8
T = 32
OPB, AB, LB, NOPS = 2, 5, 5, 4
G = 8
NCORES = 8
BC = B // NCORES          # 512 batch rows per core
P = 128
BLK = BC // P             # 4 row-blocks, batched in the free dim
COLS = NR * NB + T * OPB + 3 * T * AB + LB   # 805
TC = 4                    # scan steps per nb chunk
NCH = T // TC             # 8 chunks

# column offsets in the (host-permuted) weight matrix
OFF_R, OFF_OP, OFF_A, OFF_L = 0, 256, 320, 800

f32 = mybir.dt.float32
f16 = mybir.dt.float16
AX = mybir.AxisListType
OP = mybir.AluOpType
AF = mybir.ActivationFunctionType

_STATE = {}


def _build(repeat=1, mode='full'):
    nc = bacc.Bacc("TRN2", target_bir_lowering=False, debug=False,
                   num_devices=NCORES)
    z_d = nc.declare_dram_parameter("z", [BC, HID], f32, isOutput=False)
    wc_d = nc.declare_dram_parameter("wcat", [HID, COLS], f32, isOutput=False)
    pw_d = nc.declare_dram_parameter("pw", [P, COLS], f16, isOutput=False)
    tg_d = nc.declare_dram_parameter("tgg", [P, T * G], f16, isOutput=False)
    id_d = nc.declare_dram_parameter("ident", [P, P], f32, isOutput=False)
    idh_d = nc.declare_dram_parameter("identh", [P, P], f16, isOutput=False)
    w2_d = nc.declare_dram_parameter("w2tb", [NR + 1, HID], f16, isOutput=False)
    lg_d = nc.declare_dram_parameter("lng", [P, HID], f16, isOutput=False)
    lb_d = nc.declare_dram_parameter("lnb", [P, HID], f16, isOutput=False)
    out_d = nc.declare_dram_parameter("out", [BC, G * HID], f16, isOutput=True)

    delta = np.linspace(-1.0, 1.0, G).astype(np.float32)

    with TileContext(nc) as tc:
        with tc.tile_pool(name="cp", bufs=1) as cp, \
             tc.tile_pool(name="dp", bufs=1) as dp, \
             tc.tile_pool(name="fp", bufs=1) as fp, \
             tc.tile_pool(name="stp", bufs=2) as stp, \
             tc.tile_pool(name="scp", bufs=1) as scp, \
             tc.tile_pool(name="nbp", bufs=2) as nbp, \
             tc.tile_pool(name="ndp", bufs=3) as ndp, \
             tc.tile_pool(name="lnp", bufs=1) as lnp, \
             tc.psum_pool(name="ptr", bufs=1) as ptr, \
             tc.psum_pool(name="plg", bufs=2) as plg, \
             tc.psum_pool(name="prp", bufs=1) as prp, \
             tc.psum_pool(name="pln", bufs=2) as pln:
            tt = nc.vector.tensor_tensor
            ts = nc.vector.tensor_scalar

            # ---- constants ----
            wc = cp.tile([P, 4, COLS], f32, tag="wc")
            nc.sync.dma_start(wc[:], wc_d[:].rearrange("(k p) c -> p k c", k=4))
            pwr = cp.tile([P, COLS], f16, tag="pw")
            nc.sync.dma_start(pwr[:], pw_d[:])
            tgg = cp.tile([P, T, G], f16, tag="tgg")
            nc.sync.dma_start(tgg[:].rearrange("p t g -> p (t g)"), tg_d[:])
            ident = cp.tile([P, P], f32, tag="id")
            nc.sync.dma_start(ident[:], id_d[:])
            identh = cp.tile([P, P], f16, tag="idh")
            nc.sync.dma_start(identh[:], idh_d[:])
            w2tb = cp.tile([NR + 1, HID], f16, tag="w2")
            nc.sync.dma_start(w2tb[:], w2_d[:])
            lngr = cp.tile([P, HID], f16, tag="lng")
            nc.sync.dma_start(lngr[:], lg_d[:])
            lnbr = cp.tile([P, HID], f16, tag="lnb")
            nc.sync.dma_start(lnbr[:], lb_d[:])
            # bias constants: 8 deltas, erf offsets, eps, -k opcodes, -r regs
            bt = cp.tile([P, 16 + NR], f32, tag="bt")
            for i in range(G):
                nc.vector.memset(bt[:, i:i + 1], float(delta[i]))
            nc.vector.memset(bt[:, 8:9], 0.5)
            nc.vector.memset(bt[:, 9:10], float(NR) - 0.5)
            nc.vector.memset(bt[:, 10:11], float(NOPS) - 0.5)
            nc.vector.memset(bt[:, 11:12], 1e-5)
            for k in range(NOPS):
                nc.vector.memset(bt[:, 12 + k:13 + k], float(-k))
            for r in range(NR):
                nc.vector.memset(bt[:, 16 + r:17 + r], float(-r))

            # scan-side layouts (T outermost so chunk slices are contiguous):
            #   dv   [P, T, BLK, 3, G]      decimal addresses (s1, s2, dst)
            #   opd  [P, T, BLK, G]         decimal opcode
            #   iZ   [P, T, BLK, 3, G]      1/Z address softmax denominators
            #   coefT[P, 4, T, BLK, G]      res coefs for dots [v1n,lvn,v2n,dvn]
            #   cRM  [P, 2, T, BLK, G]      gate coefs for (R, M) planes
            #   S    [P, 2, NR, BLK, G]     register/memory state
            #   nb   [P, NR, TC, BLK, 3, G] chunk of softmax numerators

            def decode_block(blk, S, dv, opd, plen):
                """Decode one 128-row block: logits -> per-candidate decimal
                fields, written straight into the scan-layout tiles."""
                r0, r1 = blk * P, (blk + 1) * P
                zb = dp.tile([P, HID], f32, tag="zb")
                nc.sync.dma_start(zb[:], z_d[r0:r1, :])
                zt = dp.tile([P, 4, P], f32, tag="zt")
                for k in range(4):
                    tp = ptr.tile([P, P], f32, tag="tp")
                    nc.tensor.transpose(tp[:], zb[:, k * P:(k + 1) * P],
                                        ident[:])
                    nc.scalar.activation(zt[:, k, :], tp[:], AF.Copy)
                l1 = plg.tile([P, 512], f32, tag="l1")
                l2 = plg.tile([P, COLS - 512], f32, tag="l2")
                for k in range(4):
                    nc.tensor.matmul(l1[:], zt[:, k, :], wc[:, k, 0:512],
                                     start=(k == 0), stop=(k == 3))
                for k in range(4):
                    nc.tensor.matmul(l2[:], zt[:, k, :], wc[:, k, 512:COLS],
                                     start=(k == 0), stop=(k == 3))

                # per-candidate sigmoid (bias = delta_g), weighted by bit powers
                lg = dp.tile([P, COLS], f32, tag="lg")
                nc.scalar.activation(lg[:, 0:512], l1[:], AF.Copy)
                nc.scalar.activation(lg[:, 512:COLS], l2[:], AF.Copy)
                sigw = dp.tile([P, G, COLS], f16, tag="sigw")
                for g in range(G):
                    nc.scalar.activation(sigw[:, g, :], lg[:], AF.Sigmoid,
                                         bias=bt[:, g:g + 1])
                tt(sigw[:], sigw[:],
                   pwr[:].unsqueeze(1).broadcast_to([P, G, COLS]), OP.mult)

                # registers: [32r x 8b] bit-minor tree -> S R-plane
                R4 = sigw[:, :, OFF_R:OFF_OP].rearrange(
                    "p g (r b) -> p g r b", r=NR)
                r1t = dp.tile([P, G, NR, 4], f16, tag="r1t")
                tt(r1t[:], R4[:, :, :, 0:4], R4[:, :, :, 4:8], OP.add)
                r2t = dp.tile([P, G, NR, 2], f16, tag="r2t")
                tt(r2t[:], r1t[:, :, :, 0:2], r1t[:, :, :, 2:4], OP.add)
                tt(S[:, 0, :, blk, :].transpose([0, 2, 1]),
                   r2t[:, :, :, 0], r2t[:, :, :, 1], OP.add)

                # opcode: [2b x 32t] bit-major, single fold -> opd
                O2 = sigw[:, :, OFF_OP:OFF_A].rearrange(
                    "p g (b t) -> p g b t", b=OPB)
                tt(opd[:, :, blk, :].transpose([0, 2, 1]),
                   O2[:, :, 0, :], O2[:, :, 1, :], OP.add)

                # addresses: 3 fields x [5b x 32t] bit-major, serial adds
                A5 = sigw[:, :, OFF_A:OFF_L].rearrange(
                    "p g (a b t) -> p g a b t", a=3, b=AB)
                acc = dp.tile([P, G, 3, T], f16, tag="acc")
                tt(acc[:], A5[:, :, :, 0, :], A5[:, :, :, 1, :], OP.add)
                tt(acc[:], acc[:], A5[:, :, :, 2, :], OP.add)
                tt(acc[:], acc[:], A5[:, :, :, 3, :], OP.add)
                tt(dv[:, :, :, blk, :].transpose([0, 3, 2, 1]),
                   acc[:], A5[:, :, :, 4, :], OP.add)

                # program length: 5 bits, reduce
                with nc.allow_low_precision(reason="5-term fp16 sum"):
                    nc.vector.tensor_reduce(
                        plen[:, blk, :], sigw[:, :, OFF_L:COLS], AX.X, OP.add)

            def alloc_pass_tiles():
                S = stp.tile([P, 2, NR, BLK, G], f16, tag="S")
                dv = stp.tile([P, T, 3, BLK, G], f16, tag="dv")
                opd = stp.tile([P, T, BLK, G], f16, tag="opd")
                plen = stp.tile([P, BLK, G], f16, tag="plen")
                return dict(S=S, dv=dv, opd=opd, plen=plen)

            def front_coef(tl):
                """Candidate-independent coefficients (after decode)."""
                S, dv, opd, plen = tl["S"], tl["dv"], tl["opd"], tl["plen"]
                nc.vector.memset(S[:, 1], 0.0)

                # softmax denominators via erf closed form
                zf = fp.tile([P, T, 3, BLK, G], f16, tag="zf")
                z2 = fp.tile([P, T, 3, BLK, G], f16, tag="z2")
                nc.scalar.activation(zf[:], dv[:], AF.Erf, bias=bt[:, 8:9])
                nc.scalar.activation(z2[:], dv[:], AF.Erf,
                                     bias=bt[:, 9:10], scale=-1.0)
                tt(zf[:], zf[:], z2[:], OP.add)
                iZ = fp.tile([P, T, 3, BLK, G], f16, tag="iZ")
                with nc.allow_low_precision(reason="fp16 softmax denom"):
                    nc.vector.reciprocal(iZ[:], zf[:])
                zo = fp.tile([P, T, BLK, G], f16, tag="zo")
                zo2 = fp.tile([P, T, BLK, G], f16, tag="zo2")
                nc.scalar.activation(zo[:], opd[:], AF.Erf, bias=bt[:, 8:9])
                nc.scalar.activation(zo2[:], opd[:], AF.Erf,
                                     bias=bt[:, 10:11], scale=-1.0)
                tt(zo[:], zo[:], zo2[:], OP.add)
                iZop = fp.tile([P, T, BLK, G], f16, tag="iZop")
                with nc.allow_low_precision(reason="fp16 softmax denom"):
                    nc.vector.reciprocal(iZop[:], zo[:])

                # op softmax numerators, one DErf per opcode
                obx = fp.tile([P, NOPS, T, BLK, G], f16, tag="obx")
                for k in range(NOPS):
                    nc.scalar.activation(obx[:, k], opd[:],
                                         AF.Derivative_Erf,
                                         bias=bt[:, 12 + k:13 + k])

                # soft halting mask
                actx = fp.tile([P, T, BLK, G], f16, tag="actx")
                tt(actx[:], plen[:].unsqueeze(1).broadcast_to([P, T, BLK, G]),
                   tgg[:].unsqueeze(2).broadcast_to([P, T, BLK, G]),
                   OP.subtract)
                nc.scalar.activation(actx[:], actx[:], AF.Sigmoid)

                # fold denominators into per-step coefficients
                # coefT kinds [A,C,B,D] pair with dots [v1n,lvn,v2n,dvn]
                coefT = fp.tile([P, 4, T, BLK, G], f16, tag="coefT")
                cRM = fp.tile([P, 2, T, BLK, G], f16, tag="cRM")
                iZ1 = iZ[:, :, 0, :, :]
                iZ2 = iZ[:, :, 1, :, :]
                iZd = iZ[:, :, 2, :, :]
                t1 = fp.tile([P, T, BLK, G], f16, tag="t1")
                t2 = fp.tile([P, T, BLK, G], f16, tag="t2")
                tt(t1[:], obx[:, 0], obx[:, 1], OP.add)
                tt(t1[:], t1[:], iZop[:], OP.mult)
                tt(coefT[:, 0], t1[:], iZ1, OP.mult)          # A
                tt(t1[:], obx[:, 0], obx[:, 1], OP.subtract)
                tt(t1[:], t1[:], iZop[:], OP.mult)
                tt(coefT[:, 2], t1[:], iZ2, OP.mult)          # B
                tt(t1[:], obx[:, 2], iZop[:], OP.mult)
                tt(coefT[:, 1], t1[:], iZ1, OP.mult)          # C
                tt(t2[:], obx[:, 3], iZop[:], OP.mult)
                tt(coefT[:, 3], t2[:], iZd, OP.mult)          # D
                tt(cRM[:, 1], coefT[:, 3], actx[:], OP.mult)  # cM
                ts(t2[:], t2[:], -1.0, 1.0, OP.mult, OP.add)
                tt(t2[:], t2[:], iZd, OP.mult)
                tt(cRM[:, 0], t2[:], actx[:], OP.mult)        # cR
                tl["iZ"], tl["coefT"], tl["cRM"] = iZ, coefT, cRM

            def chunk_gen(c, dv):
                """nb chunk for steps [c*TC, (c+1)*TC): DErf(dv - r) with the
                -r offset as the activation bias, one contiguous write per r."""
                nb = nbp.tile([P, NR, TC, 3, BLK, G], f16, tag="nb")
                dvc = dv[:, c * TC:(c + 1) * TC, :, :, :]
                for r in range(NR):
                    nc.scalar.activation(nb[:, r], dvc, AF.Derivative_Erf,
                                         bias=bt[:, 16 + r:17 + r])
                return nb

            def scan(tl, nxt=None):
                do_dve = mode not in ("noscan",)
                do_pool = mode not in ("noscan", "nopool")
                do_chunk = mode not in ("nochunk",)
                S, dv = tl["S"], tl["dv"]
                iZ, coefT, cRM = tl["iZ"], tl["coefT"], tl["cRM"]
                q4 = scp.tile([P, 4, NR, BLK, G], f16, tag="q4")
                ndc = scp.tile([P, 2, NR, BLK, G], f16, tag="ndc")
                vbuf = scp.tile([P, 4, BLK, G], f16, tag="vbuf")
                resP = scp.tile([P, 4, BLK, G], f16, tag="resP")
                targ = scp.tile([P, 2, BLK, G], f16, tag="targ")
                S2 = [P, 2, NR, BLK, G]
                nb = tl["nb0"]
                dec_at = {3: 0, 10: 1, 17: 2, 24: 3} if nxt else {}
                for t in range(T):
                    c, tc = t // TC, t % TC
                    if tc == 0 and t > 0 and do_chunk:
                        nb = chunk_gen(c, dv)
                    if t in dec_at:
                        decode_block(dec_at[t], nxt["S"], nxt["dv"],
                                     nxt["opd"], nxt["plen"])
                    nb1 = nb[:, :, tc, 0, :, :]
                    if not do_dve:
                        continue
                    # gate tensor (DVE: HW gpsimd is too slow for this)
                    tt(ndc[:],
                       nb[:, :, tc, 2, :, :].unsqueeze(1).broadcast_to(S2),
                       cRM[:, :, t, :, :].unsqueeze(2).broadcast_to(S2),
                       OP.mult)
                    # soft gathers
                    tt(q4[:, 0:2], S[:],
                       nb1.unsqueeze(1).broadcast_to(S2), OP.mult)
                    tt(q4[:, 2], S[:, 0], nb[:, :, tc, 1, :, :], OP.mult)
                    tt(q4[:, 3], S[:, 0], nb[:, :, tc, 2, :, :], OP.mult)
                    tt(q4[:, :, 0:16], q4[:, :, 0:16], q4[:, :, 16:32],
                       OP.add)
                    tt(q4[:, :, 0:8], q4[:, :, 0:8], q4[:, :, 8:16], OP.add)
                    tt(q4[:, :, 0:4], q4[:, :, 0:4], q4[:, :, 4:8], OP.add)
                    tt(q4[:, :, 0:2], q4[:, :, 0:2], q4[:, :, 2:4], OP.add)
                    tt(vbuf[:], q4[:, :, 0], q4[:, :, 1], OP.add)
                    # res and per-plane targets
                    tt(resP[:], vbuf[:], coefT[:, :, t, :, :], OP.mult)
                    with nc.allow_low_precision(reason="4-term fp16 sum"):
                        nc.vector.tensor_reduce(
                            targ[:, 0], resP[:].transpose([0, 2, 3, 1]),
                            AX.X, OP.add)
                    tt(targ[:, 1], vbuf[:, 0], iZ[:, t, 0, :, :], OP.mult)
                    # state update: S -= ndc * (S - targ)
                    tt(q4[:, 0:2], S[:],
                       targ[:].unsqueeze(2).broadcast_to(S2), OP.subtract)
                    tt(q4[:, 2:4], q4[:, 0:2], ndc[:], OP.mult)
                    tt(S[:], S[:], q4[:, 2:4], OP.subtract)

            def ln_tail(tl):
                """register2hidden + LayerNorm + store, per block."""
                S = tl["S"]
                rft = lnp.tile([NR + 1, P], f16, tag="rft", bufs=2)
                nc.vector.memset(rft[NR:NR + 1, :], 1.0)
                rft2 = lnp.tile([NR + 1, P], f16, tag="rft2", bufs=2)
                nc.vector.memset(rft2[NR:NR + 1, :], 1.0)
                rfts = [rft, rft2]
                for blk in range(BLK):
                    r0, r1 = blk * P, (blk + 1) * P
                    hG = lnp.tile([P, G, HID], f16, tag="hG", bufs=2)
                    hsum = lnp.tile([P, G], f32, tag="hsum")
                    vsum = lnp.tile([P, G], f32, tag="vsum")
                    negmu = lnp.tile([P, G], f32, tag="negmu")
                    srt = lnp.tile([P, G], f32, tag="srt")
                    rstd = lnp.tile([P, G], f32, tag="rstd")
                    for g in range(G):
                        rp = prp.tile([NR, P], f16, tag="rp")
                        nc.tensor.transpose(rp[:], S[:, 0, :, blk, g],
                                            identh[:])
                        rf = rfts[g % 2]
                        nc.scalar.activation(rf[0:NR, :], rp[:], AF.Copy)
                        hp = pln.tile([P, HID], f32, tag="hp")
                        nc.tensor.matmul(hp[:], rf[:], w2tb[:], start=True,
                                         stop=True)
                        nc.scalar.activation(hG[:, g, :], hp[:], AF.Copy,
                                             accum_out=hsum[:, g:g + 1])
                    nc.vector.tensor_scalar_mul(negmu[:], hsum[:], -1.0 / HID)
                    for g in range(G):
                        nc.vector.tensor_scalar_add(hG[:, g, :], hG[:, g, :],
                                                    negmu[:, g:g + 1])
                    for g in range(G):
                        hsq = lnp.tile([P, HID], f16, tag="hsq")
                        nc.scalar.activation(hsq[:], hG[:, g, :], AF.Square,
                                             accum_out=vsum[:, g:g + 1])
                    nc.scalar.activation(srt[:], vsum[:], AF.Sqrt,
                                         bias=bt[:, 11:12], scale=1.0 / HID)
                    nc.vector.reciprocal(rstd[:], srt[:])
                    for g in range(G):
                        nc.vector.tensor_scalar_mul(hG[:, g, :], hG[:, g, :],
                                                    rstd[:, g:g + 1])
                    tt(hG[:], hG[:],
                       lngr[:].unsqueeze(1).broadcast_to([P, G, HID]),
                       OP.mult)
                    tt(hG[:], hG[:],
                       lnbr[:].unsqueeze(1).broadcast_to([P, G, HID]),
                       OP.add)
                    nc.sync.dma_start(
                        out_d[r0:r1, :], hG[:].rearrange("p g h -> p (g h)"))

            # pipeline: coef(k); scan(k) with decode(k+1) interleaved; ln(k)
            tl = alloc_pass_tiles()
            for blk in range(BLK):
                decode_block(blk, tl["S"], tl["dv"], tl["opd"], tl["plen"])
            for k in range(repeat):
                front_coef(tl)
                tl["nb0"] = chunk_gen(0, tl["dv"])
                nxt = alloc_pass_tiles() if k + 1 < repeat else None
                scan(tl, nxt)
                ln_tail(tl)
                tl = nxt

    nc.compile()
    return nc


def _get_nc(repeat=1, mode='full'):
    key = f"nc{repeat}_{mode}"
    if key not in _STATE:
        _STATE[key] = _build(repeat, mode)
    return _STATE[key]


def _make_consts(inputs):
    f = lambda a: np.ascontiguousarray(np.asarray(a), dtype=np.float32)

    def bit_major(w):  # [HID, T*bits] t-major -> bit-major columns
        b = w.shape[1] // T
        return w.reshape(HID, T, b).transpose(0, 2, 1).reshape(HID, T * b)

    wcat = np.concatenate([
        f(inputs["W_R"]),
        bit_major(f(inputs["W_op"])),
        bit_major(f(inputs["W_src1"])),
        bit_major(f(inputs["W_src2"])),
        bit_major(f(inputs["W_dst"])),
        f(inputs["W_len"]),
    ], axis=1)
    pw8 = (2.0 ** np.arange(NB)).astype(np.float32)
    pw2 = np.repeat(2.0 ** np.arange(OPB), T).astype(np.float32)
    pw5 = np.repeat(2.0 ** np.arange(AB), T).astype(np.float32)
    pwl = (2.0 ** np.arange(LB)).astype(np.float32)
    pw = np.concatenate([np.tile(pw8, NR), pw2, pw5, pw5, pw5, pwl])
    tgg = np.repeat(np.arange(T, dtype=np.float32) + 0.5, G)
    w2tb = np.vstack([f(inputs["W_r2h"]).T, f(inputs["b_r2h"])[None]])
    rep = lambda row: np.ascontiguousarray(np.tile(row[None], (P, 1)))
    return {
        "wcat": np.ascontiguousarray(wcat),
        "pw": rep(pw).astype(np.float16),
        "tgg": rep(tgg).astype(np.float16),
        "ident": np.eye(P, dtype=np.float32),
        "identh": np.eye(P, dtype=np.float16),
        "w2tb": np.ascontiguousarray(w2tb).astype(np.float16),
        "lng": rep(f(inputs["ln_g"])).astype(np.float16),
        "lnb": rep(f(inputs["ln_b"])).astype(np.float16),
    }


def kernel(**inputs) -> np.ndarray:
    nc = _get_nc()
    z = np.ascontiguousarray(np.asarray(inputs["z_hidden"]), dtype=np.float32)
    consts = _make_consts(inputs)
    in_maps = [dict(z=np.ascontiguousarray(z[c * BC:(c + 1) * BC]), **consts)
               for c in range(NCORES)]
    res = run_bass_kernel_spmd(nc, in_maps, list(range(NCORES)))
    out = np.concatenate(
        [np.asarray(res.results[c]["out"]) for c in range(NCORES)], axis=0)
    return out.astype(np.float32).reshape(B, G, HID)
